# revision 16
# baseline (speedup 1.0000x reference)
"""Trainium2 Bass kernel for a transformer decoder layer (B=4,S=1024,D=1024,H=16,DFF=4096).

Sharding: 8 shards = (batch, seq-half). Each NeuronCore computes its 512 output
rows end-to-end from full per-batch inputs -- no collectives.

Layout: feature-major activations (X.T: [D partitions, tokens free]); weights
pre-transposed host-side; bf16 matmul operands, f32 PSUM accumulation, f32
residual stream. Causal masking in permuted token order (own tokens first):
lower-triangular bf16 masks for the own half; the other half's all-or-nothing
visibility is folded into the exp as a -30/0 per-core bias. Softmax without
max-subtraction (scores bounded); normalization deferred to post-PV scaling;
prob-sums via an appended ones-column in the PV stationary operand.

Schedule: attention software-pipelined with PV lagging scores by 2 kk-pairs so
the PE never waits on the exp->mask chain; per-head denominators reciprocated
on DVE (no ACT table switch); layernorm stats matmuls interleaved into the
producing out-proj / FFN loops; Rsqrt for rstd (single ACT table set per LN).
"""

import sys
import types

import numpy as np
import ml_dtypes

import concourse.bass as bass
import concourse.tile as tile
import concourse.mybir as mybir
from concourse.vector_clock import ScopedClock, VectorClock

AF = mybir.ActivationFunctionType
ALU = mybir.AluOpType
DT = mybir.dt
BF16 = mybir.dt.bfloat16
F32 = mybir.dt.float32

B, S, D, H, DFF = 4, 1024, 1024, 16, 4096
DK = D // H            # 64
P = 128
SQ = S // 2            # 512 own tokens per core
NT_D = D // P          # 8
NT_FF = DFF // P       # 32
KT = S // P            # 8 kk tiles
KT_OWN = SQ // P       # 4 own kk tiles (permuted order: own first)
NPAIR = KT // 2        # 4 kk-tile pairs
GRP = 4                # heads interleaved per attention group
N_CORES = 8
EPS = 1e-5

# packed-constant column layout ([P, NCB] f32; element d=128*t+p at [p, t])
CB_SBQ, CB_SBK, CB_SBO = 0, 8, 16
CB_CBQ, CB_CBK, CB_CBO = 24, 32, 40
CB_FB2, CB_G1, CB_B1 = 48, 56, 64
CB_G2, CB_B2, CB_G3, CB_B3 = 72, 80, 88, 96
CB_FB1 = 104
NCB = 136

_NPBF16 = ml_dtypes.bfloat16


# ---------------------------------------------------------------------------
# environment patches (walrus drain-wait limit + NTFF profile hook)
# ---------------------------------------------------------------------------

_PATCHED = False


def _patch_env():
    global _PATCHED
    if _PATCHED:
        return
    _PATCHED = True

    # the pinned walrus rejects instructions with >1 sem wait on the exit
    # Drain; chunk the waits across multiple drain instructions.
    def _drain_and_barrier_chunked(self, tick_clock, wait_clock):
        ticks = [tick_clock.global_clock[i] for i in range(27)]
        nz = [(i, t) for i, t in enumerate(ticks) if t > 0]
        for i, t in nz:
            d = self.nc.sync.drain()
            c = VectorClock()
            c.require_at_least(i, t)
            wait_clock.add_sem_waits(d.ins, ScopedClock({None: c}))
        self.nc.all_engine_barrier()
        assert self.sems is not None
        popped = self.nc._tile_sem_poison_stack.pop()
        assert popped is self._sem_poison
        self.nc.clear_and_free_semaphores(list(self.sems.allocated().values()))
        self.nc.all_engine_barrier()

    tile.TileContext._drain_and_barrier = _drain_and_barrier_chunked

    # NTFF profile hook (container's antenv lacks axon_hooks)
    if 'antenv.axon_hooks' not in sys.modules:
        try:
            sys.path.insert(0, '/root/.axon_site')
            from trn_agent_boot.trn_boot import _ntff_profile_via_ctypes
            hook = _ntff_profile_via_ctypes('/opt/axon/libaxon_pjrt.so')
        except Exception:
            hook = None
        mod = types.ModuleType('antenv.axon_hooks')
        mod.get_axon_ntff_profile_hook = lambda: hook
        mod.set_axon_ntff_profile_hook = lambda h: None
        sys.modules['antenv.axon_hooks'] = mod

    import concourse.bass_utils as bu
    bu.upload_artifacts = lambda tmpdir: tmpdir


# ---------------------------------------------------------------------------
# kernel builder
# ---------------------------------------------------------------------------


def _split_excess_waits(nc, limit=1):
    """walrus encodes few sem waits per instruction; move extras onto
    preceding same-engine NoOps (engines execute in order, so waits on a
    preceding NoOp gate the instruction identically)."""
    import bass_rust
    n_added = 0
    for f in nc.m.functions:
        for blk in f.blocks:
            out = []
            for inst in blk.instructions:
                si = inst.sync_info
                waits = list(si.on_wait) if si and si.on_wait else []
                if len(waits) > limit:
                    extra, keep = waits[:-limit], waits[-limit:]
                    for w in extra:
                        nop = mybir.InstNoOp(
                            name=f"{inst.name}_xw{n_added}", ins=[], outs=[])
                        nop.engine = inst.engine
                        nop.sync_info = bass_rust.SyncInfo(
                            on_wait=[w], on_update=[])
                        out.append(nop)
                        n_added += 1
                    inst.sync_info = bass_rust.SyncInfo(
                        on_wait=keep, on_update=list(si.on_update or []))
                out.append(inst)
            blk.instructions = out
    return n_added


def _build():
    nc = bass.Bass("TRN2", target_bir_lowering=False, debug=False)

    def par(name, shape, dtype=BF16):
        return nc.declare_dram_parameter(
            name, list(shape), dtype, isOutput=False).ap()

    # per-core activations
    xT = par("xT", [D, S])                    # x[b].T, tokens permuted (own first)
    xownT = par("xownT", [D, SQ], F32)        # own residual stream, f32
    encT = par("encT", [D, S])                # enc_output[b].T
    mbias = par("mbias", [P, 1], F32)         # 0.0 if other half visible else -30.0
    # weights (shared across cores)
    wqT = par("wqT", [D, D]); wkT = par("wkT", [D, D]); wvT = par("wvT", [D, D])
    woT = par("woT", [D, D])
    cqT = par("cqT", [D, D]); ckT = par("ckT", [D, D]); cvT = par("cvT", [D, D])
    coT = par("coT", [D, D])
    w1s = par("w1s", [NT_FF, P, D])           # W1.T in sbuf-tile order per dff tile
    w2T = par("w2T", [DFF, D])
    # packed biases/ln-params ([P, t] layout; q biases pre-scaled 1/8,
    # out-proj biases include folded V-bias contribution W_o @ b_v)
    cbk = par("cb", [P, NCB], F32)

    out = nc.declare_dram_parameter("out", [D, SQ], F32, isOutput=True).ap()

    def tiled(ap, nt):  # [nt*128, N] dram -> [128, nt, N]
        return ap.rearrange("(t p) n -> p t n", p=P)

    with tile.TileContext(nc) as tc:
        def pool(name, bufs, space="SBUF"):
            return tc.tile_pool(name=name, bufs=bufs, space=space)

        with pool("consts", 1) as consts, pool("resid", 1) as resid, \
                pool("lnw", 1) as lnw:
            # ---- input DMAs first (transfers start immediately) ----
            cb_t = consts.tile([P, NCB], F32, name="cb_t")
            nc.sync.dma_start(out=cb_t, in_=cbk)
            mbias_t = consts.tile([P, 1], F32, name="mbias_t")
            nc.sync.dma_start(out=mbias_t, in_=mbias)

            def col(base, j=0):
                return cb_t[:, base + j:base + j + 1]

            # ---- constants ----
            ones128 = consts.tile([1, P], BF16, name="ones128")
            nc.vector.memset(ones128, 1.0)
            inv_d = consts.tile([P, 1], BF16, name="inv_d")
            nc.vector.memset(inv_d, 1.0 / D)
            eps_t = consts.tile([1, 1], F32, name="eps")
            nc.vector.memset(eps_t, EPS)
            zero_b = consts.tile([P, 1], F32, name="zero_b")
            nc.vector.memset(zero_b, 0.0)
            # lower-triangular bf16 masks for the 4 own kk-tiles
            ones_full = consts.tile([P, SQ], BF16, name="ones_full")
            nc.vector.memset(ones_full, 1.0)
            tri_sb = consts.tile([P, KT_OWN, SQ], BF16, name="tri")
            for _kkt in range(KT_OWN):
                nc.gpsimd.affine_select(
                    out=tri_sb[:, _kkt, :], in_=ones_full,
                    pattern=[[1, SQ]], compare_op=ALU.is_ge, fill=0.0,
                    base=-(_kkt * P), channel_multiplier=-1)

            # ---- persistent residual-stream tiles (outlive CA) ----
            z2 = resid.tile([P, NT_D, SQ], F32, name="z2")   # z1 + ca
            x2 = resid.tile([P, NT_D, SQ], BF16, name="x2")  # ln2 out

            # ===========================================================
            # helpers
            # ===========================================================

            def projection(ps_pool, w_ap, src_sb, n_tok, bias_base, dst,
                           w_pool):
                """dst[:, j, g*512:...] (feature-major [P, NT_D, n_tok]) =
                W @ src (+bias). src_sb: [P, NT_D, n_tok] bf16."""
                n_grp = n_tok // SQ
                w_tiled = tiled(w_ap, NT_D)
                for j in range(NT_D):
                    wt = w_pool.tile([P, NT_D, P], BF16, tag="w", name="w")
                    nc.sync.dma_start(out=wt, in_=w_tiled[:, :, j * P:(j + 1) * P])
                    for g in range(n_grp):
                        ps = ps_pool.tile([P, SQ], F32, tag="proj_ps", name="proj_ps")
                        for k in range(NT_D):
                            nc.tensor.matmul(
                                ps, wt[:, k, :],
                                src_sb[:, k, g * SQ:(g + 1) * SQ],
                                start=(k == 0), stop=(k == NT_D - 1))
                        nc.scalar.activation(
                            out=dst[:, j, g * SQ:(g + 1) * SQ], in_=ps,
                            func=AF.Identity, bias=col(bias_base, j), scale=1.0)

            def v_projection(ps_pool, w_ap, src_sb, v_sb, w_pool):
                """v_sb: [P, KT, H, DK+1] view of padded flat tile."""
                w_tiled = tiled(w_ap, NT_D)
                for c in range(2):  # dv chunk of 512 = 8 heads
                    wt = w_pool.tile([P, NT_D, SQ], BF16, tag="wv", name="wv")
                    nc.sync.dma_start(
                        out=wt, in_=w_tiled[:, :, c * SQ:(c + 1) * SQ])
                    for tt in range(KT):
                        ps = ps_pool.tile([P, SQ], F32, tag="proj_ps", name="proj_ps")
                        for k in range(NT_D):
                            nc.tensor.matmul(
                                ps, src_sb[:, k, tt * P:(tt + 1) * P],
                                wt[:, k, :],
                                start=(k == 0), stop=(k == NT_D - 1))
                        nc.vector.tensor_copy(
                            out=v_sb[:, tt, 8 * c:8 * c + 8, 0:DK],
                            in_=ps.rearrange("p (h d) -> p h d", d=DK))

            def q_projection(ps_pool, w_ap, src_sb, q_pad, bias_base, w_pool):
                """q_pad[:, j, half, :] = (W @ src + b) / 8, head-split."""
                w_tiled = tiled(w_ap, NT_D)
                for j in range(NT_D):
                    wt = w_pool.tile([P, NT_D, P], BF16, tag="w", name="w")
                    nc.sync.dma_start(
                        out=wt, in_=w_tiled[:, :, j * P:(j + 1) * P])
                    ps = ps_pool.tile([P, SQ], F32, tag="proj_ps",
                                      name="proj_ps")
                    for k in range(NT_D):
                        nc.tensor.matmul(ps, wt[:, k, :], src_sb[:, k, :],
                                         start=(k == 0),
                                         stop=(k == NT_D - 1))
                    nc.scalar.activation(
                        out=q_pad[0:DK, j, 0, :], in_=ps[0:DK, :],
                        func=AF.Identity,
                        bias=col(bias_base, j)[0:DK], scale=1.0 / 8.0)
                    nc.scalar.activation(
                        out=q_pad[DK:P, j, 1, :], in_=ps[DK:P, :],
                        func=AF.Identity,
                        bias=col(bias_base, j)[DK:P], scale=1.0 / 8.0)

            def attention(ph, k_sb, v_flat, q_pad, attn_sb, causal):
                """Software-pipelined attention: PV lags scores by 2 pairs;
                score psum buffers alternate per (pair, head) unit so the PE
                runs ahead of the exp stream. Per-head normalization uses the
                idle upper 64 psum rows of that head's PV bank for the
                reciprocal broadcast (no extra bank, no ACT reciprocal).
                Writes normalized head outputs to attn_sb [P, NT_D, SQ]."""
                sc_ps, pv_ps, probs, small = ph

                def _normalize(h, pv, rcb, raw):
                    # broadcast 1/Z into rows 64:128 of the PV bank (idle);
                    # DVE may read only one PSUM operand, so the raw head
                    # output was evacuated to SBUF at group end
                    dt_, off = h // 2, (h % 2) * DK
                    nc.tensor.matmul(pv[DK:P, :], ones128[:, 0:P - DK], rcb,
                                     start=True, stop=True)
                    nc.vector.tensor_mul(
                        attn_sb[off:off + DK, dt_, :], raw,
                        pv[DK:DK + DK, :])

                pend = []          # deferred per-head normalize from last group
                unit = 0           # running (pair, head) unit counter
                for h0 in range(0, H, GRP):
                    hs = list(range(h0, h0 + GRP))
                    pvs = {}
                    for h in hs:
                        pvs[h] = pv_ps.tile(
                            [P, SQ], F32,
                            tag=f"pv{h % GRP}", name=f"pv{h % GRP}")
                    prs = {}
                    for p in range(NPAIR + 2):
                        if p < NPAIR:
                            for h in hs:
                                dt_ = h // 2
                                ps = sc_ps.tile([P, 2, SQ], F32,
                                                tag=f"sc{unit % 2}",
                                                name=f"sc{unit % 2}")
                                unit += 1
                                for i in range(2):
                                    kkt = 2 * p + i
                                    nc.tensor.matmul(
                                        ps[:, i, :],
                                        k_sb[:, dt_,
                                             kkt * P:(kkt + 1) * P],
                                        q_pad[:, dt_, h % 2, :],
                                        start=True, stop=True)
                                pr = probs.tile([P, 2, SQ], BF16,
                                                tag=f"pr{h % GRP}",
                                                name="pr")
                                eb = mbias_t if (causal and p >= 2) else zero_b
                                nc.scalar.activation(out=pr, in_=ps,
                                                     func=AF.Exp, bias=eb)
                                if causal and p < 2:
                                    nc.vector.tensor_mul(
                                        pr, pr,
                                        tri_sb[:, 2 * p:2 * p + 2, :])
                                prs[(p, h)] = pr
                        if p == 1 and pend:
                            # previous group's normalize: all inputs ready,
                            # so these cost the PE no wait
                            for args in pend:
                                _normalize(*args)
                            pend = []
                        pq = p - 2
                        if pq >= 0:
                            for h in hs:
                                pr = prs.pop((pq, h))
                                for i in range(2):
                                    kkt = 2 * pq + i
                                    nc.tensor.matmul(
                                        pvs[h],
                                        v_flat[:, kkt,
                                               h * (DK + 1):
                                               h * (DK + 1) + P],
                                        pr[:, i, :],
                                        start=(kkt == 0),
                                        stop=(kkt == KT - 1))
                    # denominators: DVE reciprocal straight out of PSUM;
                    # raw head output evacuated to SBUF for the normalize
                    for h in hs:
                        rc = small.tile([1, SQ], F32, tag=f"rc{h % GRP}",
                                        name="rc")
                        nc.vector.reciprocal(rc, pvs[h][DK:DK + 1, :])
                        rcb = small.tile([1, SQ], BF16, tag=f"rb{h % GRP}",
                                         name="rcb")
                        nc.vector.tensor_copy(out=rcb, in_=rc)
                        raw = small.tile([DK, SQ], BF16,
                                         tag=f"raw{h % GRP}", name="raw")
                        nc.vector.tensor_copy(out=raw, in_=pvs[h][0:DK, :])
                        pend.append((h, pvs[h], rcb, raw))
                for args in pend:
                    _normalize(*args)

            def _normalize(h, pv, rcb):
                dt_, off = h // 2, (h % 2) * DK
                rp = norm_ps.tile([DK, SQ], F32, tag=f"rp{h % GRP}",
                                  name="rp")
                nc.tensor.matmul(rp, ones128[:, 0:DK], rcb,
                                 start=True, stop=True)
                nc.vector.tensor_mul(
                    attn_sb[off:off + DK, dt_, :], pv[0:DK, :], rp)

            def out_proj_ln(w_ap, bias_base, attn_sb, res_in, z_out,
                            g_base, b_base, x_out, wp, ops, st_ps, sqp, sm,
                            rep_ps):
                """z = res + W@attn + b; x_out = LN(z)*g + b with stats
                matmuls interleaved (lag 2) into the j loop."""
                w_tiled = tiled(w_ap, NT_D)
                mean_ps = st_ps.tile([1, SQ], F32, tag="mean", name="mean")
                sq_ps = st_ps.tile([1, SQ], F32, tag="sqm", name="sqm")
                zbs, sqs = {}, {}

                def stats(j):
                    nc.tensor.matmul(mean_ps, inv_d, zbs[j],
                                     start=(j == 0), stop=(j == NT_D - 1))
                    nc.tensor.matmul(sq_ps, inv_d, sqs[j],
                                     start=(j == 0), stop=(j == NT_D - 1))

                for j in range(NT_D):
                    wt = wp.tile([P, NT_D, P], BF16, tag="w", name="w")
                    nc.sync.dma_start(
                        out=wt, in_=w_tiled[:, :, j * P:(j + 1) * P])
                    ps = ops.tile([P, SQ], F32, tag="o_ps", name="o_ps")
                    for k in range(NT_D):
                        nc.tensor.matmul(ps, wt[:, k, :], attn_sb[:, k, :],
                                         start=(k == 0), stop=(k == NT_D - 1))
                    o = sqp.tile([P, SQ], F32, tag="o_out", name="o_out")
                    nc.scalar.activation(out=o, in_=ps, func=AF.Identity,
                                         bias=col(bias_base, j), scale=1.0)
                    nc.vector.tensor_add(z_out[:, j, :], res_in[:, j, :], o)
                    zb = sqp.tile([P, SQ], BF16, tag="zb", name="zb")
                    nc.scalar.activation(out=zb, in_=z_out[:, j, :],
                                         func=AF.Identity)
                    sq = sqp.tile([P, SQ], BF16, tag="sq", name="sq")
                    nc.vector.tensor_mul(sq, z_out[:, j, :], z_out[:, j, :])
                    zbs[j], sqs[j] = zb, sq
                    if j >= 2:
                        stats(j - 2)
                stats(NT_D - 2)
                stats(NT_D - 1)
                ln_apply(mean_ps, sq_ps, z_out, g_base, b_base, x_out,
                         sqp, sm, rep_ps)

            def ln_apply(mean_ps, sq_ps, z_sb, g_base, b_base, dst, sqp, sm,
                         rep_ps):
                mu_sb = sm.tile([1, SQ], F32, tag="mu_sb", name="mu_sb")
                nc.vector.tensor_copy(out=mu_sb, in_=mean_ps)
                mu2 = sm.tile([1, SQ], F32, tag="mu2", name="mu2")
                nc.vector.tensor_mul(mu2, mu_sb, mean_ps)
                var = sm.tile([1, SQ], F32, tag="var", name="var")
                nc.vector.tensor_sub(var, sq_ps, mu2)
                std = sm.tile([1, SQ], F32, tag="std", name="std")
                nc.scalar.activation(out=std, in_=var, func=AF.Sqrt,
                                     bias=eps_t, scale=1.0)
                rstd_f = sm.tile([1, SQ], F32, tag="rstd_f", name="rstd_f")
                nc.vector.reciprocal(rstd_f, std)
                rstd = sm.tile([1, SQ], BF16, tag="rstd", name="rstd")
                nc.vector.tensor_copy(out=rstd, in_=rstd_f)
                negmu = sm.tile([1, SQ], BF16, tag="negmu", name="negmu")
                nc.vector.tensor_scalar_mul(negmu, mean_ps, -1.0)
                rep_a = rep_ps.tile([P, SQ], F32, tag="repa", name="repa")
                nc.tensor.matmul(rep_a, ones128, rstd, start=True, stop=True)
                rep_b = rep_ps.tile([P, SQ], F32, tag="repb", name="repb")
                nc.tensor.matmul(rep_b, ones128, negmu, start=True, stop=True)
                for j in range(NT_D):
                    t1 = sqp.tile([P, SQ], F32, tag="t1", name="t1")
                    nc.vector.tensor_add(t1, z_sb[:, j, :], rep_b)
                    t2 = sqp.tile([P, SQ], F32, tag="t2", name="t2")
                    nc.vector.tensor_mul(t2, t1, rep_a)
                    nc.scalar.activation(
                        out=dst[:, j, :], in_=t2, func=AF.Identity,
                        bias=col(b_base, j), scale=col(g_base, j))

            if True:
                with pool("resA", 1) as resA, pool("eload", 1) as ep:
                    xown_sb = resA.tile([P, NT_D, SQ], F32, name="xown")
                    nc.sync.dma_start(out=xown_sb, in_=tiled(xownT, NT_D))
                    z1 = resA.tile([P, NT_D, SQ], F32, name="z1")
                    x1 = resA.tile([P, NT_D, SQ], BF16, name="x1")
                    # enc activations: loaded during SA attention, used by CA
                    e_sb = ep.tile([P, NT_D, S], BF16, name="e_sb")

                    # shared SA/CA attention workspace (memsets paid once)
                    with pool("wksp", 1) as wksp:
                        k_sb = wksp.tile([P, NT_D, S], BF16, name="k_sb")
                        v_flat = wksp.tile(
                            [P, KT, H * (DK + 1) + P - (DK + 1)],
                            BF16, name="v_flat")
                        v_sb = v_flat[:, :, 0:H * (DK + 1)].rearrange(
                            "p t (h d) -> p t h d", d=DK + 1)
                        q_pad = wksp.tile([P, NT_D, 2, SQ], BF16, name="q_pad")
                        attn_sb = wksp.tile([P, NT_D, SQ], BF16, name="attn_sb")

                        # =================================================
                        # Phase 1: self-attention projections
                        # =================================================
                        with pool("sa_ps", 2, "PSUM") as ps_pool, \
                                pool("sa_x", 1) as xp, pool("sa_w", 3) as wp:
                            x_sb = xp.tile([P, NT_D, S], BF16, name="x_sb")
                            nc.sync.dma_start(out=x_sb, in_=tiled(xT, NT_D))
                            # zero-padding for q and the v ones-columns: off
                            # the DMA critical path, runs during projections
                            nc.vector.memset(q_pad, 0.0)
                            nc.vector.memset(v_flat[:, :, H * (DK + 1):], 0.0)
                            for tt in range(KT):
                                nc.vector.memset(v_sb[:, tt, :, DK:DK + 1],
                                                 1.0)
                            projection(ps_pool, wkT, x_sb, S, CB_SBK, k_sb,
                                       wp)
                            v_projection(ps_pool, wvT, x_sb, v_sb, wp)
                            q_projection(ps_pool, wqT, x_sb[:, :, 0:SQ],
                                         q_pad, CB_SBQ, wp)

                        # enc DMA issues here; transfer overlaps SA attention
                        nc.sync.dma_start(out=e_sb, in_=tiled(encT, NT_D))

                        with pool("sa_sc", 1, "PSUM") as sc_ps, \
                                pool("sa_pv", 1, "PSUM") as pv_ps, \
                                pool("sa_pr", 3) as probs, \
                                pool("sa_sm", 1) as small:
                            attention((sc_ps, pv_ps, probs, small),
                                      k_sb, v_flat, q_pad, attn_sb, True)

                        # out proj + residual + LN1 (stats interleaved)
                        with pool("sa_ops", 2, "PSUM") as ops, \
                                pool("sa_st", 1, "PSUM") as st_ps, \
                                pool("sa_rep", 1, "PSUM") as rep_ps, \
                                pool("sa_wo", 3) as wp2, \
                                pool("sa_sq", 3) as sqp, pool("sa_sm2", 1) as sm:
                            out_proj_ln(woT, CB_SBO, attn_sb, xown_sb, z1,
                                        CB_G1, CB_B1, x1, wp2, ops, st_ps,
                                        sqp, sm, rep_ps)

                        # =================================================
                        # Phase 2: cross-attention
                        # =================================================
                        with pool("ca_ps", 2, "PSUM") as ps_pool, \
                                pool("ca_w", 3) as wp:
                            projection(ps_pool, ckT, e_sb, S, CB_CBK, k_sb,
                                       wp)
                            v_projection(ps_pool, cvT, e_sb, v_sb, wp)
                            q_projection(ps_pool, cqT, x1, q_pad, CB_CBQ, wp)

                        with pool("ca_sc", 1, "PSUM") as sc_ps, \
                                pool("ca_pv", 1, "PSUM") as pv_ps, \
                                pool("ca_pr", 3) as probs, \
                                pool("ca_sm", 1) as small:
                            attention((sc_ps, pv_ps, probs, small),
                                      k_sb, v_flat, q_pad, attn_sb, False)

                        with pool("ca_ops", 2, "PSUM") as ops, \
                                pool("ca_st", 1, "PSUM") as st_ps, \
                                pool("ca_rep", 1, "PSUM") as rep_ps, \
                                pool("ca_wo", 3) as wp2, \
                                pool("ca_sq", 3) as sqp, pool("ca_sm2", 1) as sm:
                            out_proj_ln(coT, CB_CBO, attn_sb, z1, z2,
                                        CB_G2, CB_B2, x2, wp2, ops, st_ps,
                                        sqp, sm, rep_ps)

                # ===========================================================
                # Phase 3: FFN (+ LN3 stats interleaved into the w2 loop)
                # ===========================================================
                with pool("ff_h", 1) as hp, pool("ff_w1", 4) as w1p, \
                        pool("ff_w2", 1) as w2p, \
                        pool("ff_ps", 2, "PSUM") as ps_pool, \
                        pool("ff_st", 1, "PSUM") as st_ps, \
                        pool("ff_rep", 1, "PSUM") as rep_ps, \
                        pool("ff_sq", 3) as sqp, pool("ff_sm", 1) as sm, \
                        pool("out_p", 2) as outp:
                    h_sb = hp.tile([P, NT_FF, SQ], BF16, name="h_sb")
                    w2_sb = w2p.tile([P, NT_FF, D], BF16, name="w2_sb")
                    nc.sync.dma_start(out=w2_sb, in_=tiled(w2T, NT_FF))
                    for f in range(NT_FF):
                        wt = w1p.tile([P, NT_D, P], BF16, tag="w1", name="w1")
                        nc.sync.dma_start(out=wt, in_=w1s[f])
                        ps = ps_pool.tile([P, SQ], F32, tag="h_ps", name="h_ps")
                        for k in range(NT_D):
                            nc.tensor.matmul(ps, wt[:, k, :], x2[:, k, :],
                                             start=(k == 0), stop=(k == NT_D - 1))
                        nc.scalar.activation(
                            out=h_sb[:, f, :], in_=ps, func=AF.Relu,
                            bias=col(CB_FB1, f), scale=1.0)
                    z3 = hp.tile([P, NT_D, SQ], F32, name="z3")
                    mean_ps = st_ps.tile([1, SQ], F32, tag="mean", name="mean")
                    sq_ps = st_ps.tile([1, SQ], F32, tag="sqm", name="sqm")
                    zbs, sqs = {}, {}

                    def stats3(j):
                        nc.tensor.matmul(mean_ps, inv_d, zbs[j],
                                         start=(j == 0), stop=(j == NT_D - 1))
                        nc.tensor.matmul(sq_ps, inv_d, sqs[j],
                                         start=(j == 0), stop=(j == NT_D - 1))

                    for j in range(NT_D):
                        ps = ps_pool.tile([P, SQ], F32, tag="y_ps", name="y_ps")
                        for k in range(NT_FF):
                            nc.tensor.matmul(
                                ps, w2_sb[:, k, j * P:(j + 1) * P],
                                h_sb[:, k, :],
                                start=(k == 0), stop=(k == NT_FF - 1))
                        y = sqp.tile([P, SQ], F32, tag="ff_out", name="ff_out")
                        nc.scalar.activation(out=y, in_=ps, func=AF.Identity,
                                             bias=col(CB_FB2, j), scale=1.0)
                        nc.vector.tensor_add(z3[:, j, :], z2[:, j, :], y)
                        zb = sqp.tile([P, SQ], BF16, tag="zb", name="zb")
                        nc.scalar.activation(out=zb, in_=z3[:, j, :],
                                             func=AF.Identity)
                        sq = sqp.tile([P, SQ], BF16, tag="sq", name="sq")
                        nc.vector.tensor_mul(sq, z3[:, j, :], z3[:, j, :])
                        zbs[j], sqs[j] = zb, sq
                        if j >= 1:
                            stats3(j - 1)
                    stats3(NT_D - 1)

                    # LN3 apply -> out (f32), DMA per j
                    mu_sb = sm.tile([1, SQ], F32, tag="mu_sb", name="mu_sb")
                    nc.vector.tensor_copy(out=mu_sb, in_=mean_ps)
                    mu2 = sm.tile([1, SQ], F32, tag="mu2", name="mu2")
                    nc.vector.tensor_mul(mu2, mu_sb, mean_ps)
                    var = sm.tile([1, SQ], F32, tag="var", name="var")
                    nc.vector.tensor_sub(var, sq_ps, mu2)
                    std = sm.tile([1, SQ], F32, tag="std", name="std")
                    nc.scalar.activation(out=std, in_=var, func=AF.Sqrt,
                                         bias=eps_t, scale=1.0)
                    rstd_f = sm.tile([1, SQ], F32, tag="rstd_f", name="rstd_f")
                    nc.vector.reciprocal(rstd_f, std)
                    rstd = sm.tile([1, SQ], BF16, tag="rstd", name="rstd")
                    nc.vector.tensor_copy(out=rstd, in_=rstd_f)
                    negmu = sm.tile([1, SQ], BF16, tag="negmu", name="negmu")
                    nc.vector.tensor_scalar_mul(negmu, mean_ps, -1.0)
                    rep_a = rep_ps.tile([P, SQ], F32, tag="repa", name="repa")
                    nc.tensor.matmul(rep_a, ones128, rstd, start=True,
                                     stop=True)
                    rep_b = rep_ps.tile([P, SQ], F32, tag="repb", name="repb")
                    nc.tensor.matmul(rep_b, ones128, negmu, start=True,
                                     stop=True)
                    for j in range(NT_D):
                        t1 = sqp.tile([P, SQ], F32, tag="t1", name="t1")
                        nc.vector.tensor_add(t1, z3[:, j, :], rep_b)
                        t2 = sqp.tile([P, SQ], F32, tag="t2", name="t2")
                        nc.vector.tensor_mul(t2, t1, rep_a)
                        yo = outp.tile([P, SQ], F32, tag="yo", name="yo")
                        nc.scalar.activation(
                            out=yo, in_=t2, func=AF.Identity,
                            bias=col(CB_B3, j), scale=col(CB_G3, j))
                        nc.sync.dma_start(
                            out=tiled(out, NT_D)[:, j, :], in_=yo)

    _split_excess_waits(nc)
    return nc


# ---------------------------------------------------------------------------
# host wrapper
# ---------------------------------------------------------------------------

_NC_CACHE = {}
_TRACE = False          # set kernel._TRACE = True to profile (exec_time_ns)
_LAST_RESULT = None     # BassKernelResults of the last run


def _get_nc():
    if "nc" not in _NC_CACHE:
        _patch_env()
        _NC_CACHE["nc"] = _build()
    return _NC_CACHE["nc"]


def _bf16(a):
    return np.ascontiguousarray(np.asarray(a, np.float32)).astype(_NPBF16)


def _bias_pack(v, nt):
    return np.ascontiguousarray(
        np.asarray(v, np.float32).reshape(nt, P).T).astype(np.float32)


def kernel(x, enc_output, source_mask, target_mask,
           sa_wq, sa_bq, sa_wk, sa_bk, sa_wv, sa_bv, sa_wo, sa_bo,
           ca_in_w, ca_in_b, ca_out_w, ca_out_b,
           ff_w1, ff_b1, ff_w2, ff_b2,
           n1_g, n1_b, n2_g, n2_b, n3_g, n3_b):
    from concourse.bass_utils import run_bass_kernel_spmd

    nc = _get_nc()
    x = np.asarray(x, np.float32)
    enc = np.asarray(enc_output, np.float32)

    ca_in_w = np.asarray(ca_in_w, np.float32)
    ca_in_b = np.asarray(ca_in_b, np.float32)
    wq_c, wk_c, wv_c = ca_in_w[:D], ca_in_w[D:2 * D], ca_in_w[2 * D:]
    bq_c, bk_c, bv_c = ca_in_b[:D], ca_in_b[D:2 * D], ca_in_b[2 * D:]

    # fold V bias through the out-projection: (attn + bv) @ Wo.T + bo
    #   = attn @ Wo.T + (Wo @ bv + bo)
    sbo_f = np.asarray(sa_bo, np.float32) + \
        np.asarray(sa_wo, np.float32) @ np.asarray(sa_bv, np.float32)
    cbo_f = np.asarray(ca_out_b, np.float32) + \
        np.asarray(ca_out_w, np.float32) @ np.asarray(bv_c, np.float32)

    cb = np.zeros((P, NCB), np.float32)
    cb[:, CB_SBQ:CB_SBQ + NT_D] = _bias_pack(np.asarray(sa_bq) / 8.0, NT_D)
    cb[:, CB_SBK:CB_SBK + NT_D] = _bias_pack(sa_bk, NT_D)
    cb[:, CB_SBO:CB_SBO + NT_D] = _bias_pack(sbo_f, NT_D)
    cb[:, CB_CBQ:CB_CBQ + NT_D] = _bias_pack(bq_c / 8.0, NT_D)
    cb[:, CB_CBK:CB_CBK + NT_D] = _bias_pack(bk_c, NT_D)
    cb[:, CB_CBO:CB_CBO + NT_D] = _bias_pack(cbo_f, NT_D)
    cb[:, CB_FB2:CB_FB2 + NT_D] = _bias_pack(ff_b2, NT_D)
    cb[:, CB_G1:CB_G1 + NT_D] = _bias_pack(n1_g, NT_D)
    cb[:, CB_B1:CB_B1 + NT_D] = _bias_pack(n1_b, NT_D)
    cb[:, CB_G2:CB_G2 + NT_D] = _bias_pack(n2_g, NT_D)
    cb[:, CB_B2:CB_B2 + NT_D] = _bias_pack(n2_b, NT_D)
    cb[:, CB_G3:CB_G3 + NT_D] = _bias_pack(n3_g, NT_D)
    cb[:, CB_B3:CB_B3 + NT_D] = _bias_pack(n3_b, NT_D)
    cb[:, CB_FB1:CB_FB1 + NT_FF] = _bias_pack(ff_b1, NT_FF)

    shared = {
        "wqT": _bf16(np.asarray(sa_wq).T), "wkT": _bf16(np.asarray(sa_wk).T),
        "wvT": _bf16(np.asarray(sa_wv).T), "woT": _bf16(np.asarray(sa_wo).T),
        "cqT": _bf16(wq_c.T), "ckT": _bf16(wk_c.T), "cvT": _bf16(wv_c.T),
        "coT": _bf16(np.asarray(ca_out_w).T),
        "w2T": _bf16(np.asarray(ff_w2).T),
        "cb": cb,
    }
    # W1.T in per-dff-tile sbuf order: [NT_FF][P, NT_D, P] -> [NT_FF, P, NT_D*P]
    w1T = _bf16(np.asarray(ff_w1).T)  # [D, DFF]
    w1r = w1T.reshape(NT_D, P, NT_FF, P)  # [kt, p, ft, pf]
    w1s = np.ascontiguousarray(
        w1r.transpose(2, 1, 0, 3).reshape(NT_FF, P, NT_D * P))
    shared["w1s"] = w1s

    in_maps = []
    for c in range(N_CORES):
        b, half = c // 2, c % 2
        own = slice(half * SQ, half * SQ + SQ)
        other = slice((1 - half) * SQ, (1 - half) * SQ + SQ)
        xTb = x[b].T  # [D, S]
        xperm = np.concatenate([xTb[:, own], xTb[:, other]], axis=1)
        m = dict(shared)
        m["xT"] = _bf16(xperm)
        m["xownT"] = np.ascontiguousarray(xTb[:, own]).astype(np.float32)
        m["encT"] = _bf16(enc[b].T)
        m["mbias"] = np.full((P, 1), 0.0 if half else -30.0, np.float32)
        in_maps.append(m)

    global _LAST_RESULT
    res = run_bass_kernel_spmd(nc, in_maps, core_ids=list(range(N_CORES)),
                               trace=_TRACE)
    _LAST_RESULT = res
    out = np.empty((B, S, D), np.float32)
    for c in range(N_CORES):
        b, half = c // 2, c % 2
        out[b, half * SQ:half * SQ + SQ, :] = res.results[c]["out"].T
    return out


# revision 18
# speedup vs baseline: 1.0009x; 1.0009x over previous
"""Trainium2 Bass kernel for a transformer decoder layer (B=4,S=1024,D=1024,H=16,DFF=4096).

Sharding: 8 shards = (batch, seq-half). Each NeuronCore computes its 512 output
rows end-to-end from full per-batch inputs -- no collectives.

Layout: feature-major activations (X.T: [D partitions, tokens free]); weights
pre-transposed host-side; bf16 matmul operands, f32 PSUM accumulation, f32
residual stream. Causal masking in permuted token order (own tokens first):
lower-triangular bf16 masks for the own half; the other half's all-or-nothing
visibility is folded into the exp as a -30/0 per-core bias. Softmax without
max-subtraction (scores bounded); normalization deferred to post-PV scaling;
prob-sums via an appended ones-column in the PV stationary operand.

Schedule: attention software-pipelined with PV lagging scores by 2 kk-pairs so
the PE never waits on the exp->mask chain; per-head denominators reciprocated
on DVE (no ACT table switch); layernorm stats matmuls interleaved into the
producing out-proj / FFN loops; Rsqrt for rstd (single ACT table set per LN).
"""

import sys
import types

import numpy as np
import ml_dtypes

import concourse.bass as bass
import concourse.tile as tile
import concourse.mybir as mybir
from concourse.vector_clock import ScopedClock, VectorClock

AF = mybir.ActivationFunctionType
ALU = mybir.AluOpType
DT = mybir.dt
BF16 = mybir.dt.bfloat16
F32 = mybir.dt.float32

B, S, D, H, DFF = 4, 1024, 1024, 16, 4096
DK = D // H            # 64
P = 128
SQ = S // 2            # 512 own tokens per core
NT_D = D // P          # 8
NT_FF = DFF // P       # 32
KT = S // P            # 8 kk tiles
KT_OWN = SQ // P       # 4 own kk tiles (permuted order: own first)
NPAIR = KT // 2        # 4 kk-tile pairs
GRP = 4                # heads interleaved per attention group
N_CORES = 8
EPS = 1e-5

# packed-constant column layout ([P, NCB] f32; element d=128*t+p at [p, t])
CB_SBQ, CB_SBK, CB_SBO = 0, 8, 16
CB_CBQ, CB_CBK, CB_CBO = 24, 32, 40
CB_FB2, CB_G1, CB_B1 = 48, 56, 64
CB_G2, CB_B2, CB_G3, CB_B3 = 72, 80, 88, 96
CB_FB1 = 104
NCB = 136

_NPBF16 = ml_dtypes.bfloat16


# ---------------------------------------------------------------------------
# environment patches (walrus drain-wait limit + NTFF profile hook)
# ---------------------------------------------------------------------------

_PATCHED = False


def _patch_env():
    global _PATCHED
    if _PATCHED:
        return
    _PATCHED = True

    # the pinned walrus rejects instructions with >1 sem wait on the exit
    # Drain; chunk the waits across multiple drain instructions.
    def _drain_and_barrier_chunked(self, tick_clock, wait_clock):
        ticks = [tick_clock.global_clock[i] for i in range(27)]
        nz = [(i, t) for i, t in enumerate(ticks) if t > 0]
        for i, t in nz:
            d = self.nc.sync.drain()
            c = VectorClock()
            c.require_at_least(i, t)
            wait_clock.add_sem_waits(d.ins, ScopedClock({None: c}))
        self.nc.all_engine_barrier()
        assert self.sems is not None
        popped = self.nc._tile_sem_poison_stack.pop()
        assert popped is self._sem_poison
        self.nc.clear_and_free_semaphores(list(self.sems.allocated().values()))
        self.nc.all_engine_barrier()

    tile.TileContext._drain_and_barrier = _drain_and_barrier_chunked

    # NTFF profile hook (container's antenv lacks axon_hooks)
    if 'antenv.axon_hooks' not in sys.modules:
        try:
            sys.path.insert(0, '/root/.axon_site')
            from trn_agent_boot.trn_boot import _ntff_profile_via_ctypes
            hook = _ntff_profile_via_ctypes('/opt/axon/libaxon_pjrt.so')
        except Exception:
            hook = None
        mod = types.ModuleType('antenv.axon_hooks')
        mod.get_axon_ntff_profile_hook = lambda: hook
        mod.set_axon_ntff_profile_hook = lambda h: None
        sys.modules['antenv.axon_hooks'] = mod

    import concourse.bass_utils as bu
    bu.upload_artifacts = lambda tmpdir: tmpdir


# ---------------------------------------------------------------------------
# kernel builder
# ---------------------------------------------------------------------------


def _split_excess_waits(nc, limit=1):
    """walrus encodes few sem waits per instruction; move extras onto
    preceding same-engine NoOps (engines execute in order, so waits on a
    preceding NoOp gate the instruction identically)."""
    import bass_rust
    n_added = 0
    for f in nc.m.functions:
        for blk in f.blocks:
            out = []
            for inst in blk.instructions:
                si = inst.sync_info
                waits = list(si.on_wait) if si and si.on_wait else []
                if len(waits) > limit:
                    extra, keep = waits[:-limit], waits[-limit:]
                    for w in extra:
                        nop = mybir.InstNoOp(
                            name=f"{inst.name}_xw{n_added}", ins=[], outs=[])
                        nop.engine = inst.engine
                        nop.sync_info = bass_rust.SyncInfo(
                            on_wait=[w], on_update=[])
                        out.append(nop)
                        n_added += 1
                    inst.sync_info = bass_rust.SyncInfo(
                        on_wait=keep, on_update=list(si.on_update or []))
                out.append(inst)
            blk.instructions = out
    return n_added


def _build():
    nc = bass.Bass("TRN2", target_bir_lowering=False, debug=False)

    def par(name, shape, dtype=BF16):
        return nc.declare_dram_parameter(
            name, list(shape), dtype, isOutput=False).ap()

    # per-core activations
    xT = par("xT", [D, S])                    # x[b].T, tokens permuted (own first)
    xownT = par("xownT", [D, SQ], F32)        # own residual stream, f32
    encT = par("encT", [D, S])                # enc_output[b].T
    mbias = par("mbias", [P, 1], F32)         # 0.0 if other half visible else -30.0
    # weights (shared across cores)
    wqT = par("wqT", [D, D]); wkT = par("wkT", [D, D]); wvT = par("wvT", [D, D])
    woT = par("woT", [D, D])
    cqT = par("cqT", [D, D]); ckT = par("ckT", [D, D]); cvT = par("cvT", [D, D])
    coT = par("coT", [D, D])
    w1s = par("w1s", [NT_FF, P, D])           # W1.T in sbuf-tile order per dff tile
    w2T = par("w2T", [DFF, D])
    # packed biases/ln-params ([P, t] layout; q biases pre-scaled 1/8,
    # out-proj biases include folded V-bias contribution W_o @ b_v)
    cbk = par("cb", [P, NCB], F32)

    out = nc.declare_dram_parameter("out", [D, SQ], F32, isOutput=True).ap()

    def tiled(ap, nt):  # [nt*128, N] dram -> [128, nt, N]
        return ap.rearrange("(t p) n -> p t n", p=P)

    with tile.TileContext(nc) as tc:
        def pool(name, bufs, space="SBUF"):
            return tc.tile_pool(name=name, bufs=bufs, space=space)

        with pool("consts", 1) as consts, pool("resid", 1) as resid, \
                pool("lnw", 1) as lnw:
            # ---- input DMAs first (transfers start immediately) ----
            cb_t = consts.tile([P, NCB], F32, name="cb_t")
            nc.sync.dma_start(out=cb_t, in_=cbk)
            mbias_t = consts.tile([P, 1], F32, name="mbias_t")
            nc.sync.dma_start(out=mbias_t, in_=mbias)

            def col(base, j=0):
                return cb_t[:, base + j:base + j + 1]

            # ---- constants ----
            ones128 = consts.tile([1, P], BF16, name="ones128")
            nc.vector.memset(ones128, 1.0)
            inv_d = consts.tile([P, 1], BF16, name="inv_d")
            nc.vector.memset(inv_d, 1.0 / D)
            eps_t = consts.tile([1, 1], F32, name="eps")
            nc.vector.memset(eps_t, EPS)
            zero_b = consts.tile([P, 1], F32, name="zero_b")
            nc.vector.memset(zero_b, 0.0)
            # lower-triangular bf16 masks for the 4 own kk-tiles
            ones_full = consts.tile([P, SQ], BF16, name="ones_full")
            nc.vector.memset(ones_full, 1.0)
            tri_sb = consts.tile([P, KT_OWN, SQ], BF16, name="tri")
            for _kkt in range(KT_OWN):
                nc.gpsimd.affine_select(
                    out=tri_sb[:, _kkt, :], in_=ones_full,
                    pattern=[[1, SQ]], compare_op=ALU.is_ge, fill=0.0,
                    base=-(_kkt * P), channel_multiplier=-1)

            # ---- persistent residual-stream tiles (outlive CA) ----
            z2 = resid.tile([P, NT_D, SQ], F32, name="z2")   # z1 + ca
            x2 = resid.tile([P, NT_D, SQ], BF16, name="x2")  # ln2 out

            # ===========================================================
            # helpers
            # ===========================================================

            def projection(ps_pool, w_ap, src_sb, n_tok, bias_base, dst,
                           w_pool):
                """dst[:, j, g*512:...] (feature-major [P, NT_D, n_tok]) =
                W @ src (+bias). src_sb: [P, NT_D, n_tok] bf16."""
                n_grp = n_tok // SQ
                w_tiled = tiled(w_ap, NT_D)
                for j in range(NT_D):
                    wt = w_pool.tile([P, NT_D, P], BF16, tag="w", name="w")
                    nc.sync.dma_start(out=wt, in_=w_tiled[:, :, j * P:(j + 1) * P])
                    for g in range(n_grp):
                        ps = ps_pool.tile([P, SQ], F32, tag="proj_ps", name="proj_ps")
                        for k in range(NT_D):
                            nc.tensor.matmul(
                                ps, wt[:, k, :],
                                src_sb[:, k, g * SQ:(g + 1) * SQ],
                                start=(k == 0), stop=(k == NT_D - 1))
                        nc.scalar.activation(
                            out=dst[:, j, g * SQ:(g + 1) * SQ], in_=ps,
                            func=AF.Identity, bias=col(bias_base, j), scale=1.0)

            def v_projection(ps_pool, w_ap, src_sb, v_sb, w_pool):
                """v_sb: [P, KT, H, DK+1] view of padded flat tile."""
                w_tiled = tiled(w_ap, NT_D)
                for c in range(2):  # dv chunk of 512 = 8 heads
                    wt = w_pool.tile([P, NT_D, SQ], BF16, tag="wv", name="wv")
                    nc.sync.dma_start(
                        out=wt, in_=w_tiled[:, :, c * SQ:(c + 1) * SQ])
                    for tt in range(KT):
                        ps = ps_pool.tile([P, SQ], F32, tag="proj_ps", name="proj_ps")
                        for k in range(NT_D):
                            nc.tensor.matmul(
                                ps, src_sb[:, k, tt * P:(tt + 1) * P],
                                wt[:, k, :],
                                start=(k == 0), stop=(k == NT_D - 1))
                        nc.vector.tensor_copy(
                            out=v_sb[:, tt, 8 * c:8 * c + 8, 0:DK],
                            in_=ps.rearrange("p (h d) -> p h d", d=DK))

            def q_projection(ps_pool, w_ap, src_sb, q_pad, bias_base, w_pool):
                """q_pad[:, j, half, :] = (W @ src + b) / 8, head-split."""
                w_tiled = tiled(w_ap, NT_D)
                for j in range(NT_D):
                    wt = w_pool.tile([P, NT_D, P], BF16, tag="w", name="w")
                    nc.sync.dma_start(
                        out=wt, in_=w_tiled[:, :, j * P:(j + 1) * P])
                    ps = ps_pool.tile([P, SQ], F32, tag="proj_ps",
                                      name="proj_ps")
                    for k in range(NT_D):
                        nc.tensor.matmul(ps, wt[:, k, :], src_sb[:, k, :],
                                         start=(k == 0),
                                         stop=(k == NT_D - 1))
                    nc.scalar.activation(
                        out=q_pad[0:DK, j, 0, :], in_=ps[0:DK, :],
                        func=AF.Identity,
                        bias=col(bias_base, j)[0:DK], scale=1.0 / 8.0)
                    nc.scalar.activation(
                        out=q_pad[DK:P, j, 1, :], in_=ps[DK:P, :],
                        func=AF.Identity,
                        bias=col(bias_base, j)[DK:P], scale=1.0 / 8.0)

            def attention(ph, k_sb, v_flat, q_pad, attn_sb, causal):
                """Software-pipelined attention: PV lags scores by 2 pairs;
                score psum buffers alternate per (pair, head) unit so the PE
                runs ahead of the exp stream. Per-head normalization uses the
                idle upper 64 psum rows of that head's PV bank for the
                reciprocal broadcast (no extra bank, no ACT reciprocal).
                Writes normalized head outputs to attn_sb [P, NT_D, SQ]."""
                sc_ps, pv_ps, probs, small = ph

                def _normalize(h, pv, rcb, raw):
                    # broadcast 1/Z into rows 64:128 of the PV bank (idle);
                    # DVE may read only one PSUM operand, so the raw head
                    # output was evacuated to SBUF at group end
                    dt_, off = h // 2, (h % 2) * DK
                    nc.tensor.matmul(pv[DK:P, :], ones128[:, 0:P - DK], rcb,
                                     start=True, stop=True)
                    nc.vector.tensor_mul(
                        attn_sb[off:off + DK, dt_, :], raw,
                        pv[DK:DK + DK, :])

                pend = []          # deferred per-head normalize from last group
                unit = 0           # running (pair, head) unit counter
                for h0 in range(0, H, GRP):
                    hs = list(range(h0, h0 + GRP))
                    pvs = {}
                    for h in hs:
                        pvs[h] = pv_ps.tile(
                            [P, SQ], F32,
                            tag=f"pv{h % GRP}", name=f"pv{h % GRP}")
                    prs = {}
                    for p in range(NPAIR + 2):
                        if p < NPAIR:
                            for h in hs:
                                dt_ = h // 2
                                ps = sc_ps.tile([P, 2, SQ], F32,
                                                tag=f"sc{unit % 2}",
                                                name=f"sc{unit % 2}")
                                unit += 1
                                for i in range(2):
                                    kkt = 2 * p + i
                                    nc.tensor.matmul(
                                        ps[:, i, :],
                                        k_sb[:, dt_,
                                             kkt * P:(kkt + 1) * P],
                                        q_pad[:, dt_, h % 2, :],
                                        start=True, stop=True)
                                pr = probs.tile([P, 2, SQ], BF16,
                                                tag=f"pr{h % GRP}",
                                                name="pr")
                                eb = mbias_t if (causal and p >= 2) else zero_b
                                nc.scalar.activation(out=pr, in_=ps,
                                                     func=AF.Exp, bias=eb)
                                if causal and p < 2:
                                    nc.vector.tensor_mul(
                                        pr, pr,
                                        tri_sb[:, 2 * p:2 * p + 2, :])
                                prs[(p, h)] = pr
                        if p == 1 and pend:
                            # previous group's normalize: all inputs ready,
                            # so these cost the PE no wait
                            for args in pend:
                                _normalize(*args)
                            pend = []
                        pq = p - 2
                        if pq >= 0:
                            for h in hs:
                                pr = prs.pop((pq, h))
                                for i in range(2):
                                    kkt = 2 * pq + i
                                    nc.tensor.matmul(
                                        pvs[h],
                                        v_flat[:, kkt,
                                               h * (DK + 1):
                                               h * (DK + 1) + P],
                                        pr[:, i, :],
                                        start=(kkt == 0),
                                        stop=(kkt == KT - 1))
                    # denominators: DVE reciprocal straight out of PSUM;
                    # raw head output evacuated to SBUF for the normalize
                    for h in hs:
                        rc = small.tile([1, SQ], F32, tag=f"rc{h % GRP}",
                                        name="rc")
                        nc.vector.reciprocal(rc, pvs[h][DK:DK + 1, :])
                        rcb = small.tile([1, SQ], BF16, tag=f"rb{h % GRP}",
                                         name="rcb")
                        nc.scalar.activation(out=rcb, in_=rc,
                                             func=AF.Identity)
                        raw = small.tile([DK, SQ], BF16,
                                         tag=f"raw{h % GRP}", name="raw")
                        nc.scalar.activation(out=raw, in_=pvs[h][0:DK, :],
                                             func=AF.Identity)
                        pend.append((h, pvs[h], rcb, raw))
                for args in pend:
                    _normalize(*args)

            def _normalize(h, pv, rcb):
                dt_, off = h // 2, (h % 2) * DK
                rp = norm_ps.tile([DK, SQ], F32, tag=f"rp{h % GRP}",
                                  name="rp")
                nc.tensor.matmul(rp, ones128[:, 0:DK], rcb,
                                 start=True, stop=True)
                nc.vector.tensor_mul(
                    attn_sb[off:off + DK, dt_, :], pv[0:DK, :], rp)

            def out_proj_ln(w_ap, bias_base, attn_sb, res_in, z_out,
                            g_base, b_base, x_out, wp, ops, st_ps, sqp, sm,
                            rep_ps):
                """z = res + W@attn + b; x_out = LN(z)*g + b with stats
                matmuls interleaved (lag 2) into the j loop."""
                w_tiled = tiled(w_ap, NT_D)
                mean_ps = st_ps.tile([1, SQ], F32, tag="mean", name="mean")
                sq_ps = st_ps.tile([1, SQ], F32, tag="sqm", name="sqm")
                zbs, sqs = {}, {}

                def stats(j):
                    nc.tensor.matmul(mean_ps, inv_d, zbs[j],
                                     start=(j == 0), stop=(j == NT_D - 1))
                    nc.tensor.matmul(sq_ps, inv_d, sqs[j],
                                     start=(j == 0), stop=(j == NT_D - 1))

                for j in range(NT_D):
                    wt = wp.tile([P, NT_D, P], BF16, tag="w", name="w")
                    nc.sync.dma_start(
                        out=wt, in_=w_tiled[:, :, j * P:(j + 1) * P])
                    ps = ops.tile([P, SQ], F32, tag="o_ps", name="o_ps")
                    for k in range(NT_D):
                        nc.tensor.matmul(ps, wt[:, k, :], attn_sb[:, k, :],
                                         start=(k == 0), stop=(k == NT_D - 1))
                    o = sqp.tile([P, SQ], F32, tag="o_out", name="o_out")
                    nc.scalar.activation(out=o, in_=ps, func=AF.Identity,
                                         bias=col(bias_base, j), scale=1.0)
                    nc.vector.tensor_add(z_out[:, j, :], res_in[:, j, :], o)
                    zb = sqp.tile([P, SQ], BF16, tag="zb", name="zb")
                    nc.scalar.activation(out=zb, in_=z_out[:, j, :],
                                         func=AF.Identity)
                    sq = sqp.tile([P, SQ], BF16, tag="sq", name="sq")
                    nc.vector.tensor_mul(sq, z_out[:, j, :], z_out[:, j, :])
                    zbs[j], sqs[j] = zb, sq
                    if j >= 2:
                        stats(j - 2)
                stats(NT_D - 2)
                stats(NT_D - 1)
                ln_apply(mean_ps, sq_ps, z_out, g_base, b_base, x_out,
                         sqp, sm, rep_ps)

            def ln_apply(mean_ps, sq_ps, z_sb, g_base, b_base, dst, sqp, sm,
                         rep_ps):
                mu_sb = sm.tile([1, SQ], F32, tag="mu_sb", name="mu_sb")
                nc.vector.tensor_copy(out=mu_sb, in_=mean_ps)
                mu2 = sm.tile([1, SQ], F32, tag="mu2", name="mu2")
                nc.vector.tensor_mul(mu2, mu_sb, mean_ps)
                var = sm.tile([1, SQ], F32, tag="var", name="var")
                nc.vector.tensor_sub(var, sq_ps, mu2)
                std = sm.tile([1, SQ], F32, tag="std", name="std")
                nc.scalar.activation(out=std, in_=var, func=AF.Sqrt,
                                     bias=eps_t, scale=1.0)
                rstd_f = sm.tile([1, SQ], F32, tag="rstd_f", name="rstd_f")
                nc.vector.reciprocal(rstd_f, std)
                rstd = sm.tile([1, SQ], BF16, tag="rstd", name="rstd")
                nc.vector.tensor_copy(out=rstd, in_=rstd_f)
                negmu = sm.tile([1, SQ], BF16, tag="negmu", name="negmu")
                nc.vector.tensor_scalar_mul(negmu, mean_ps, -1.0)
                rep_a = rep_ps.tile([P, SQ], F32, tag="repa", name="repa")
                nc.tensor.matmul(rep_a, ones128, rstd, start=True, stop=True)
                rep_b = rep_ps.tile([P, SQ], F32, tag="repb", name="repb")
                nc.tensor.matmul(rep_b, ones128, negmu, start=True, stop=True)
                for j in range(NT_D):
                    t1 = sqp.tile([P, SQ], F32, tag="t1", name="t1")
                    nc.vector.tensor_add(t1, z_sb[:, j, :], rep_b)
                    t2 = sqp.tile([P, SQ], F32, tag="t2", name="t2")
                    nc.vector.tensor_mul(t2, t1, rep_a)
                    nc.scalar.activation(
                        out=dst[:, j, :], in_=t2, func=AF.Identity,
                        bias=col(b_base, j), scale=col(g_base, j))

            if True:
                with pool("resA", 1) as resA, pool("eload", 1) as ep:
                    xown_sb = resA.tile([P, NT_D, SQ], F32, name="xown")
                    nc.sync.dma_start(out=xown_sb, in_=tiled(xownT, NT_D))
                    z1 = resA.tile([P, NT_D, SQ], F32, name="z1")
                    x1 = resA.tile([P, NT_D, SQ], BF16, name="x1")
                    # enc activations: loaded during SA attention, used by CA
                    e_sb = ep.tile([P, NT_D, S], BF16, name="e_sb")

                    # shared SA/CA attention workspace (memsets paid once)
                    with pool("wksp", 1) as wksp:
                        k_sb = wksp.tile([P, NT_D, S], BF16, name="k_sb")
                        v_flat = wksp.tile(
                            [P, KT, H * (DK + 1) + P - (DK + 1)],
                            BF16, name="v_flat")
                        v_sb = v_flat[:, :, 0:H * (DK + 1)].rearrange(
                            "p t (h d) -> p t h d", d=DK + 1)
                        q_pad = wksp.tile([P, NT_D, 2, SQ], BF16, name="q_pad")
                        attn_sb = wksp.tile([P, NT_D, SQ], BF16, name="attn_sb")

                        # =================================================
                        # Phase 1: self-attention projections
                        # =================================================
                        with pool("sa_ps", 2, "PSUM") as ps_pool, \
                                pool("sa_x", 1) as xp, pool("sa_w", 3) as wp:
                            x_sb = xp.tile([P, NT_D, S], BF16, name="x_sb")
                            nc.sync.dma_start(out=x_sb, in_=tiled(xT, NT_D))
                            # zero-padding for q and the v ones-columns: off
                            # the DMA critical path, runs during projections
                            nc.vector.memset(q_pad, 0.0)
                            nc.vector.memset(v_flat[:, :, H * (DK + 1):], 0.0)
                            for tt in range(KT):
                                nc.vector.memset(v_sb[:, tt, :, DK:DK + 1],
                                                 1.0)
                            projection(ps_pool, wkT, x_sb, S, CB_SBK, k_sb,
                                       wp)
                            v_projection(ps_pool, wvT, x_sb, v_sb, wp)
                            q_projection(ps_pool, wqT, x_sb[:, :, 0:SQ],
                                         q_pad, CB_SBQ, wp)

                        # enc DMA issues here; transfer overlaps SA attention
                        nc.sync.dma_start(out=e_sb, in_=tiled(encT, NT_D))

                        with pool("sa_sc", 1, "PSUM") as sc_ps, \
                                pool("sa_pv", 1, "PSUM") as pv_ps, \
                                pool("sa_pr", 3) as probs, \
                                pool("sa_sm", 1) as small:
                            attention((sc_ps, pv_ps, probs, small),
                                      k_sb, v_flat, q_pad, attn_sb, True)

                        # out proj + residual + LN1 (stats interleaved)
                        with pool("sa_ops", 2, "PSUM") as ops, \
                                pool("sa_st", 1, "PSUM") as st_ps, \
                                pool("sa_rep", 1, "PSUM") as rep_ps, \
                                pool("sa_wo", 3) as wp2, \
                                pool("sa_sq", 3) as sqp, pool("sa_sm2", 1) as sm:
                            out_proj_ln(woT, CB_SBO, attn_sb, xown_sb, z1,
                                        CB_G1, CB_B1, x1, wp2, ops, st_ps,
                                        sqp, sm, rep_ps)

                        # =================================================
                        # Phase 2: cross-attention
                        # =================================================
                        with pool("ca_ps", 2, "PSUM") as ps_pool, \
                                pool("ca_w", 3) as wp:
                            projection(ps_pool, ckT, e_sb, S, CB_CBK, k_sb,
                                       wp)
                            v_projection(ps_pool, cvT, e_sb, v_sb, wp)
                            q_projection(ps_pool, cqT, x1, q_pad, CB_CBQ, wp)

                        with pool("ca_sc", 1, "PSUM") as sc_ps, \
                                pool("ca_pv", 1, "PSUM") as pv_ps, \
                                pool("ca_pr", 3) as probs, \
                                pool("ca_sm", 1) as small:
                            attention((sc_ps, pv_ps, probs, small),
                                      k_sb, v_flat, q_pad, attn_sb, False)

                        with pool("ca_ops", 2, "PSUM") as ops, \
                                pool("ca_st", 1, "PSUM") as st_ps, \
                                pool("ca_rep", 1, "PSUM") as rep_ps, \
                                pool("ca_wo", 3) as wp2, \
                                pool("ca_sq", 3) as sqp, pool("ca_sm2", 1) as sm:
                            out_proj_ln(coT, CB_CBO, attn_sb, z1, z2,
                                        CB_G2, CB_B2, x2, wp2, ops, st_ps,
                                        sqp, sm, rep_ps)

                # ===========================================================
                # Phase 3: FFN (+ LN3 stats interleaved into the w2 loop)
                # ===========================================================
                with pool("ff_h", 1) as hp, pool("ff_w1", 4) as w1p, \
                        pool("ff_w2", 1) as w2p, \
                        pool("ff_ps", 2, "PSUM") as ps_pool, \
                        pool("ff_st", 1, "PSUM") as st_ps, \
                        pool("ff_rep", 1, "PSUM") as rep_ps, \
                        pool("ff_sq", 3) as sqp, pool("ff_sm", 1) as sm, \
                        pool("out_p", 2) as outp:
                    h_sb = hp.tile([P, NT_FF, SQ], BF16, name="h_sb")
                    w2_sb = w2p.tile([P, NT_FF, D], BF16, name="w2_sb")
                    for f in range(NT_FF):
                        wt = w1p.tile([P, NT_D, P], BF16, tag="w1", name="w1")
                        nc.sync.dma_start(out=wt, in_=w1s[f])
                        if f == 4:
                            # issue the 8MB w2 load after the first w1 tiles
                            # are in flight so it doesn't starve the w1 loop
                            nc.sync.dma_start(out=w2_sb, in_=tiled(w2T, NT_FF))
                        ps = ps_pool.tile([P, SQ], F32, tag="h_ps", name="h_ps")
                        for k in range(NT_D):
                            nc.tensor.matmul(ps, wt[:, k, :], x2[:, k, :],
                                             start=(k == 0), stop=(k == NT_D - 1))
                        nc.scalar.activation(
                            out=h_sb[:, f, :], in_=ps, func=AF.Relu,
                            bias=col(CB_FB1, f), scale=1.0)
                    z3 = hp.tile([P, NT_D, SQ], F32, name="z3")
                    mean_ps = st_ps.tile([1, SQ], F32, tag="mean", name="mean")
                    sq_ps = st_ps.tile([1, SQ], F32, tag="sqm", name="sqm")
                    zbs, sqs = {}, {}

                    def stats3(j):
                        nc.tensor.matmul(mean_ps, inv_d, zbs[j],
                                         start=(j == 0), stop=(j == NT_D - 1))
                        nc.tensor.matmul(sq_ps, inv_d, sqs[j],
                                         start=(j == 0), stop=(j == NT_D - 1))

                    for j in range(NT_D):
                        ps = ps_pool.tile([P, SQ], F32, tag="y_ps", name="y_ps")
                        for k in range(NT_FF):
                            nc.tensor.matmul(
                                ps, w2_sb[:, k, j * P:(j + 1) * P],
                                h_sb[:, k, :],
                                start=(k == 0), stop=(k == NT_FF - 1))
                        y = sqp.tile([P, SQ], F32, tag="ff_out", name="ff_out")
                        nc.scalar.activation(out=y, in_=ps, func=AF.Identity,
                                             bias=col(CB_FB2, j), scale=1.0)
                        nc.vector.tensor_add(z3[:, j, :], z2[:, j, :], y)
                        zb = sqp.tile([P, SQ], BF16, tag="zb", name="zb")
                        nc.scalar.activation(out=zb, in_=z3[:, j, :],
                                             func=AF.Identity)
                        sq = sqp.tile([P, SQ], BF16, tag="sq", name="sq")
                        nc.vector.tensor_mul(sq, z3[:, j, :], z3[:, j, :])
                        zbs[j], sqs[j] = zb, sq
                        if j >= 1:
                            stats3(j - 1)
                    stats3(NT_D - 1)

                    # LN3 apply -> out (f32), DMA per j
                    mu_sb = sm.tile([1, SQ], F32, tag="mu_sb", name="mu_sb")
                    nc.vector.tensor_copy(out=mu_sb, in_=mean_ps)
                    mu2 = sm.tile([1, SQ], F32, tag="mu2", name="mu2")
                    nc.vector.tensor_mul(mu2, mu_sb, mean_ps)
                    var = sm.tile([1, SQ], F32, tag="var", name="var")
                    nc.vector.tensor_sub(var, sq_ps, mu2)
                    std = sm.tile([1, SQ], F32, tag="std", name="std")
                    nc.scalar.activation(out=std, in_=var, func=AF.Sqrt,
                                         bias=eps_t, scale=1.0)
                    rstd_f = sm.tile([1, SQ], F32, tag="rstd_f", name="rstd_f")
                    nc.vector.reciprocal(rstd_f, std)
                    rstd = sm.tile([1, SQ], BF16, tag="rstd", name="rstd")
                    nc.vector.tensor_copy(out=rstd, in_=rstd_f)
                    negmu = sm.tile([1, SQ], BF16, tag="negmu", name="negmu")
                    nc.vector.tensor_scalar_mul(negmu, mean_ps, -1.0)
                    rep_a = rep_ps.tile([P, SQ], F32, tag="repa", name="repa")
                    nc.tensor.matmul(rep_a, ones128, rstd, start=True,
                                     stop=True)
                    rep_b = rep_ps.tile([P, SQ], F32, tag="repb", name="repb")
                    nc.tensor.matmul(rep_b, ones128, negmu, start=True,
                                     stop=True)
                    for j in range(NT_D):
                        t1 = sqp.tile([P, SQ], F32, tag="t1", name="t1")
                        nc.vector.tensor_add(t1, z3[:, j, :], rep_b)
                        t2 = sqp.tile([P, SQ], F32, tag="t2", name="t2")
                        nc.vector.tensor_mul(t2, t1, rep_a)
                        yo = outp.tile([P, SQ], F32, tag="yo", name="yo")
                        nc.scalar.activation(
                            out=yo, in_=t2, func=AF.Identity,
                            bias=col(CB_B3, j), scale=col(CB_G3, j))
                        nc.sync.dma_start(
                            out=tiled(out, NT_D)[:, j, :], in_=yo)

    _split_excess_waits(nc)
    return nc


# ---------------------------------------------------------------------------
# host wrapper
# ---------------------------------------------------------------------------

_NC_CACHE = {}
_TRACE = False          # set kernel._TRACE = True to profile (exec_time_ns)
_LAST_RESULT = None     # BassKernelResults of the last run


def _get_nc():
    if "nc" not in _NC_CACHE:
        _patch_env()
        _NC_CACHE["nc"] = _build()
    return _NC_CACHE["nc"]


def _bf16(a):
    return np.ascontiguousarray(np.asarray(a, np.float32)).astype(_NPBF16)


def _bias_pack(v, nt):
    return np.ascontiguousarray(
        np.asarray(v, np.float32).reshape(nt, P).T).astype(np.float32)


def kernel(x, enc_output, source_mask, target_mask,
           sa_wq, sa_bq, sa_wk, sa_bk, sa_wv, sa_bv, sa_wo, sa_bo,
           ca_in_w, ca_in_b, ca_out_w, ca_out_b,
           ff_w1, ff_b1, ff_w2, ff_b2,
           n1_g, n1_b, n2_g, n2_b, n3_g, n3_b):
    from concourse.bass_utils import run_bass_kernel_spmd

    nc = _get_nc()
    x = np.asarray(x, np.float32)
    enc = np.asarray(enc_output, np.float32)

    ca_in_w = np.asarray(ca_in_w, np.float32)
    ca_in_b = np.asarray(ca_in_b, np.float32)
    wq_c, wk_c, wv_c = ca_in_w[:D], ca_in_w[D:2 * D], ca_in_w[2 * D:]
    bq_c, bk_c, bv_c = ca_in_b[:D], ca_in_b[D:2 * D], ca_in_b[2 * D:]

    # fold V bias through the out-projection: (attn + bv) @ Wo.T + bo
    #   = attn @ Wo.T + (Wo @ bv + bo)
    sbo_f = np.asarray(sa_bo, np.float32) + \
        np.asarray(sa_wo, np.float32) @ np.asarray(sa_bv, np.float32)
    cbo_f = np.asarray(ca_out_b, np.float32) + \
        np.asarray(ca_out_w, np.float32) @ np.asarray(bv_c, np.float32)

    cb = np.zeros((P, NCB), np.float32)
    cb[:, CB_SBQ:CB_SBQ + NT_D] = _bias_pack(np.asarray(sa_bq) / 8.0, NT_D)
    cb[:, CB_SBK:CB_SBK + NT_D] = _bias_pack(sa_bk, NT_D)
    cb[:, CB_SBO:CB_SBO + NT_D] = _bias_pack(sbo_f, NT_D)
    cb[:, CB_CBQ:CB_CBQ + NT_D] = _bias_pack(bq_c / 8.0, NT_D)
    cb[:, CB_CBK:CB_CBK + NT_D] = _bias_pack(bk_c, NT_D)
    cb[:, CB_CBO:CB_CBO + NT_D] = _bias_pack(cbo_f, NT_D)
    cb[:, CB_FB2:CB_FB2 + NT_D] = _bias_pack(ff_b2, NT_D)
    cb[:, CB_G1:CB_G1 + NT_D] = _bias_pack(n1_g, NT_D)
    cb[:, CB_B1:CB_B1 + NT_D] = _bias_pack(n1_b, NT_D)
    cb[:, CB_G2:CB_G2 + NT_D] = _bias_pack(n2_g, NT_D)
    cb[:, CB_B2:CB_B2 + NT_D] = _bias_pack(n2_b, NT_D)
    cb[:, CB_G3:CB_G3 + NT_D] = _bias_pack(n3_g, NT_D)
    cb[:, CB_B3:CB_B3 + NT_D] = _bias_pack(n3_b, NT_D)
    cb[:, CB_FB1:CB_FB1 + NT_FF] = _bias_pack(ff_b1, NT_FF)

    shared = {
        "wqT": _bf16(np.asarray(sa_wq).T), "wkT": _bf16(np.asarray(sa_wk).T),
        "wvT": _bf16(np.asarray(sa_wv).T), "woT": _bf16(np.asarray(sa_wo).T),
        "cqT": _bf16(wq_c.T), "ckT": _bf16(wk_c.T), "cvT": _bf16(wv_c.T),
        "coT": _bf16(np.asarray(ca_out_w).T),
        "w2T": _bf16(np.asarray(ff_w2).T),
        "cb": cb,
    }
    # W1.T in per-dff-tile sbuf order: [NT_FF][P, NT_D, P] -> [NT_FF, P, NT_D*P]
    w1T = _bf16(np.asarray(ff_w1).T)  # [D, DFF]
    w1r = w1T.reshape(NT_D, P, NT_FF, P)  # [kt, p, ft, pf]
    w1s = np.ascontiguousarray(
        w1r.transpose(2, 1, 0, 3).reshape(NT_FF, P, NT_D * P))
    shared["w1s"] = w1s

    in_maps = []
    for c in range(N_CORES):
        b, half = c // 2, c % 2
        own = slice(half * SQ, half * SQ + SQ)
        other = slice((1 - half) * SQ, (1 - half) * SQ + SQ)
        xTb = x[b].T  # [D, S]
        xperm = np.concatenate([xTb[:, own], xTb[:, other]], axis=1)
        m = dict(shared)
        m["xT"] = _bf16(xperm)
        m["xownT"] = np.ascontiguousarray(xTb[:, own]).astype(np.float32)
        m["encT"] = _bf16(enc[b].T)
        m["mbias"] = np.full((P, 1), 0.0 if half else -30.0, np.float32)
        in_maps.append(m)

    global _LAST_RESULT
    res = run_bass_kernel_spmd(nc, in_maps, core_ids=list(range(N_CORES)),
                               trace=_TRACE)
    _LAST_RESULT = res
    out = np.empty((B, S, D), np.float32)
    for c in range(N_CORES):
        b, half = c // 2, c % 2
        out[b, half * SQ:half * SQ + SQ, :] = res.results[c]["out"].T
    return out
